# revision 7
# baseline (speedup 1.0000x reference)
"""nn_Mixup_Branch kernel for 8 trn2 NeuronCores.

Sharding: core c -> (batch b = c//2, time half h = c%2).  Each core computes
conv1/conv2/GN/ReLU on its (batch, T-half), the inverse-CDF gather from
frame_level_feature, and conv3/GN/ReLU on the concatenated mix.  GroupNorm
statistics need full-T reductions -> two tiny pairwise AllReduces of
per-channel [sum, sumsq].

All matmuls run in float32r (fp32 storage, ~2^-12 effective PE precision).
The gather index vector (2048 int32) is computed on host with the exact jax
ops of the reference (fp32 cumsum rounding determines integer truncation
boundaries, so it must be bit-exact; a 4096-deep sequential fp32 chain is not
reproducible on-device).
"""

import numpy as np

B, C, T, TF, P = 4, 512, 2048, 4096, 512
GROUPS = 32
EPS = 1e-5
TL = T // 2  # per-core time slice
NCORES = 8

_cache = {}


def _build_nc(debug=False):
    import concourse.bacc as bacc
    import concourse.bass as bass
    import concourse.tile as tile
    import concourse.mybir as mybir

    f32 = mybir.dt.float32
    f32r = mybir.dt.float32r
    i16 = mybir.dt.int16
    AF = mybir.ActivationFunctionType
    OP = mybir.AluOpType

    nc = bacc.Bacc("TRN2", num_devices=NCORES)

    # ---- I/O ----
    x_d = nc.dram_tensor("x", [C, TL], f32r, kind="ExternalInput")
    flf_d = nc.dram_tensor("flf", [TF, C], f32r, kind="ExternalInput")
    idx_d = nc.dram_tensor("idx", [128, TL // 16], i16, kind="ExternalInput")
    w1_d = nc.dram_tensor("w1t", [C, P], f32r, kind="ExternalInput")
    w2_d = nc.dram_tensor("w2t", [C, 2 * P], f32r, kind="ExternalInput")
    w3_d = nc.dram_tensor("w3t", [4 * P, C], f32r, kind="ExternalInput")
    # packed per-channel params [bias, gamma, beta] for each conv
    pp1_d = nc.dram_tensor("pp1", [P, 3], f32, kind="ExternalInput")
    pp2_d = nc.dram_tensor("pp2", [2 * P, 3], f32, kind="ExternalInput")
    pp3_d = nc.dram_tensor("pp3", [C, 3], f32, kind="ExternalInput")
    oh_d = nc.dram_tensor("oh", [128, B], f32, kind="ExternalInput")
    adj16_d = nc.dram_tensor("adj16", [128, 128], f32, kind="ExternalInput")
    adj32_d = nc.dram_tensor("adj32", [128, 128], f32, kind="ExternalInput")
    id_d = nc.dram_tensor("idm", [128, 128], f32r, kind="ExternalInput")

    feat_d = nc.dram_tensor("feat", [2 * P, TL], f32, kind="ExternalOutput")
    mixed_d = nc.dram_tensor("mixed", [C, TL], f32, kind="ExternalOutput")
    if debug:
        smp_d = nc.dram_tensor("smp_dbg", [128, 4, TL], f32, kind="ExternalOutput")
        raw3_d = nc.dram_tensor("raw3_dbg", [128, 4, TL], f32, kind="ExternalOutput")

    NK = C // 128          # 4 contraction chunks for conv1/2
    NM1, NM2, NM3 = P // 128, 2 * P // 128, C // 128   # 4, 8, 4
    NK3 = 4 * P // 128     # 16
    NT = TL // 512         # 2 time chunks of 512

    with tile.TileContext(nc) as tc:
        import contextlib
        ctx = contextlib.ExitStack()
        sb = ctx.enter_context(tc.tile_pool(name="sb", bufs=1))
        pcv = ctx.enter_context(tc.tile_pool(name="pcv", bufs=4, space="PSUM"))
        ptp = ctx.enter_context(tc.tile_pool(name="ptp", bufs=4, space="PSUM"))
        dram = ctx.enter_context(tc.tile_pool(name="dram", bufs=1, space="DRAM"))

        # ---- loads: small/critical first, big ones chunked ----
        idxt = sb.tile([128, TL // 16], i16, tag="idx")
        nc.sync.dma_start(out=idxt[:], in_=idx_d[:])
        ident = sb.tile([128, 128], f32r, tag="idm")
        nc.sync.dma_start(out=ident[:], in_=id_d[:])
        adj16 = sb.tile([128, 128], f32, tag="adj16")
        nc.sync.dma_start(out=adj16[:], in_=adj16_d[:])
        adj32 = sb.tile([128, 128], f32, tag="adj32")
        nc.sync.dma_start(out=adj32[:], in_=adj32_d[:])
        oht = sb.tile([128, B], f32, tag="oh")
        nc.sync.dma_start(out=oht[:], in_=oh_d[:])
        pp1 = sb.tile([128, NM1, 3], f32, tag="pp1")
        nc.sync.dma_start(out=pp1[:], in_=pp1_d.rearrange("(m p) v -> p m v", p=128))
        pp2 = sb.tile([128, NM2, 3], f32, tag="pp2")
        nc.sync.dma_start(out=pp2[:], in_=pp2_d.rearrange("(m p) v -> p m v", p=128))
        pp3 = sb.tile([128, NM3, 3], f32, tag="pp3")
        nc.sync.dma_start(out=pp3[:], in_=pp3_d.rearrange("(m p) v -> p m v", p=128))
        epst = sb.tile([128, 1], f32, tag="eps")
        nc.vector.memset(epst[:], EPS)

        # ---- gather (independent of convs, gated only on idxt) ----
        gt = sb.tile([128, TL // 128, C], f32r, tag="gt")
        nc.gpsimd.dma_gather(out_ap=gt[:], in_ap=flf_d[:], idxs_ap=idxt[:],
                             num_idxs=TL, num_idxs_reg=TL, elem_size=C)

        xt = sb.tile([128, NK, TL], f32r, tag="x")
        xr = x_d.rearrange("(k p) t -> p k t", p=128)
        w1 = sb.tile([128, NK, NM1, 128], f32r, tag="w1")
        w1r = w1_d.rearrange("(k p) (m q) -> p k m q", p=128, q=128)
        for k in range(NK):
            nc.sync.dma_start(out=xt[:, k, :], in_=xr[:, k, :])
            nc.sync.dma_start(out=w1[:, k, :, :], in_=w1r[:, k, :, :])
        w2 = sb.tile([128, NK, NM2, 128], f32r, tag="w2")
        w2r = w2_d.rearrange("(k p) (m q) -> p k m q", p=128, q=128)
        for k in range(NK):
            nc.sync.dma_start(out=w2[:, k, :, :], in_=w2r[:, k, :, :])
        w3 = sb.tile([128, NK3, NM3, 128], f32r, tag="w3")
        w3r = w3_d.rearrange("(k p) (m q) -> p k m q", p=128, q=128)
        for k in range(0, NK3, 4):
            nc.sync.dma_start(out=w3[:, k:k + 4, :, :], in_=w3r[:, k:k + 4, :, :])

        # ---- conv helper: matmuls + psum->sbuf copy + per-tile bn_stats ----
        def conv(w, nk, nm, raw, st6, rhs_of_kc):
            for mo in range(nm):
                for tt in range(NT):
                    ps = pcv.tile([128, 512], f32, tag="cv")
                    for kc in range(nk):
                        nc.tensor.matmul(
                            ps[:], w[:, kc, mo, :], rhs_of_kc(kc, tt),
                            start=(kc == 0), stop=(kc == nk - 1))
                    nc.scalar.copy(out=raw[:, mo, tt * 512:(tt + 1) * 512], in_=ps[:])
                    nc.vector.bn_stats(out=st6[:, mo, tt, :],
                                       in_=raw[:, mo, tt * 512:(tt + 1) * 512].bitcast(f32))

        raw1 = sb.tile([128, NM1, TL], f32r, tag="raw1")
        st1 = sb.tile([128, NM1, NT, 6], f32, tag="st1")
        conv(w1, NK, NM1, raw1, st1, lambda kc, tt: xt[:, kc, tt * 512:(tt + 1) * 512])

        raw2 = sb.tile([128, NM2, TL], f32r, tag="raw2")
        st2 = sb.tile([128, NM2, NT, 6], f32, tag="st2")
        conv(w2, NK, NM2, raw2, st2, lambda kc, tt: xt[:, kc, tt * 512:(tt + 1) * 512])

        # ---- per-channel [sum', sumsq'] with bias folded in ----
        def chan_stats(st6, pp, nm, out_ss, tg):
            # bn_aggr per output-chunk: (128, NT, 6) -> (128, 2) [mean, var]
            mv = sb.tile([128, nm, 2], f32, tag=f"mv{tg}")
            for mo in range(nm):
                nc.vector.bn_aggr(out=mv[:, mo, :], in_=st6[:, mo, :, :])
            # sum = mean*TL + b*TL ; sumsq = (var+mean^2)*TL + 2b*sum_mm + b^2*TL
            sm_mm = sb.tile([128, nm], f32, tag=f"smm{tg}")
            nc.vector.tensor_scalar(out=sm_mm[:], in0=mv[:, :, 0], scalar1=float(TL),
                                    scalar2=None, op0=OP.mult)
            m2 = sb.tile([128, nm], f32, tag=f"m2{tg}")
            nc.vector.tensor_tensor(out=m2[:], in0=mv[:, :, 0], in1=mv[:, :, 0], op=OP.mult)
            nc.vector.tensor_tensor(out=m2[:], in0=m2[:], in1=mv[:, :, 1], op=OP.add)
            # m2 now = mean^2 + var ; sumsq_mm = m2*TL
            nc.vector.tensor_scalar(out=m2[:], in0=m2[:], scalar1=float(TL),
                                    scalar2=None, op0=OP.mult)
            # bias terms
            bcol = pp[:, :, 0]
            t0 = sb.tile([128, nm], f32, tag=f"t0{tg}")
            nc.vector.tensor_tensor(out=t0[:], in0=bcol, in1=sm_mm[:], op=OP.mult)
            nc.vector.tensor_scalar(out=t0[:], in0=t0[:], scalar1=2.0, scalar2=None, op0=OP.mult)
            nc.vector.tensor_tensor(out=m2[:], in0=m2[:], in1=t0[:], op=OP.add)
            nc.vector.tensor_tensor(out=t0[:], in0=bcol, in1=bcol, op=OP.mult)
            nc.vector.tensor_scalar(out=t0[:], in0=t0[:], scalar1=float(TL), scalar2=None, op0=OP.mult)
            nc.vector.tensor_tensor(out=m2[:], in0=m2[:], in1=t0[:], op=OP.add)
            # sum' = sum_mm + b*TL
            nc.vector.tensor_scalar(out=t0[:], in0=bcol, scalar1=float(TL), scalar2=None, op0=OP.mult)
            nc.vector.tensor_tensor(out=out_ss[:, :, 0], in0=t0[:], in1=sm_mm[:], op=OP.add)
            nc.vector.tensor_copy(out=out_ss[:, :, 1], in_=m2[:])

        NMS1 = NM1 + NM2
        ss1 = sb.tile([128, NMS1, 2], f32, tag="ss1")
        chan_stats(st1, pp1, NM1, ss1[:, 0:NM1, :], "c1")
        chan_stats(st2, pp2, NM2, ss1[:, NM1:NMS1, :], "c2")

        def allreduce_stats(ss, nms, tg):
            # pad into per-batch slots via one-hot, one 8-wide AllReduce,
            # then select own slot back out (pairwise groups serialize on
            # the collective cores; a single 8-wide group does not).
            pay = sb.tile([128, B, nms, 2], f32, tag=f"pay{tg}")
            for s in range(B):
                nc.vector.tensor_scalar(out=pay[:, s, :, :], in0=ss[:],
                                        scalar1=oht[:, s:s + 1], scalar2=None, op0=OP.mult)
            ar_in = dram.tile([128, B, nms, 2], f32)
            ar_out = dram.tile([128, B, nms, 2], f32)
            nc.sync.dma_start(out=ar_in[:], in_=pay[:])
            nc.gpsimd.collective_compute(
                "AllReduce", OP.add,
                replica_groups=[[0, 1, 2, 3, 4, 5, 6, 7]],
                ins=[ar_in.opt()], outs=[ar_out.opt()])
            back = sb.tile([128, B, nms, 2], f32, tag=f"back{tg}")
            nc.sync.dma_start(out=back[:], in_=ar_out[:])
            sel = sb.tile([128, nms, 2], f32, tag=f"sel{tg}")
            tmp = sb.tile([128, nms, 2], f32, tag=f"seltmp{tg}")
            nc.vector.tensor_scalar(out=sel[:], in0=back[:, 0, :, :],
                                    scalar1=oht[:, 0:1], scalar2=None, op0=OP.mult)
            for s in range(1, B):
                nc.vector.tensor_scalar(out=tmp[:], in0=back[:, s, :, :],
                                        scalar1=oht[:, s:s + 1], scalar2=None, op0=OP.mult)
                nc.vector.tensor_tensor(out=sel[:], in0=sel[:], in1=tmp[:], op=OP.add)
            return sel

        sel1 = allreduce_stats(ss1, NMS1, "1")

        # ---- transpose gathered rows to channel-major while AR runs ----
        smp = sb.tile([128, NK, TL], f32r, tag="smp")
        for blk in range(TL // 128):
            for kb in range(NK):
                pt = ptp.tile([128, 128], f32r, tag="tp")
                nc.tensor.transpose(pt[:], gt[:, blk, kb * 128:(kb + 1) * 128], ident[:])
                nc.vector.tensor_copy(out=smp[:, kb, blk * 128:(blk + 1) * 128], in_=pt[:])

        # ---- GN finalize: group reduce via adjacency matmul, scale/shift ----
        def gn_finalize(sel_ap, adj, pp, nm, ngel, scale, shift, tg):
            gs = sb.tile([128, nm, 2], f32, tag=f"gs{tg}")
            for mo in range(nm):
                pg = ptp.tile([128, 2], f32, tag="tp")
                nc.tensor.matmul(pg[:], adj[:], sel_ap[:, mo, :],
                                 start=True, stop=True)
                nc.scalar.copy(out=gs[:, mo, :], in_=pg[:])
            ninv = 1.0 / float(ngel * T)
            mu = sb.tile([128, nm], f32, tag=f"mu{tg}")
            nc.vector.tensor_scalar(out=mu[:], in0=gs[:, :, 0], scalar1=ninv, scalar2=None, op0=OP.mult)
            var = sb.tile([128, nm], f32, tag=f"va{tg}")
            nc.vector.tensor_scalar(out=var[:], in0=gs[:, :, 1], scalar1=ninv, scalar2=None, op0=OP.mult)
            t1 = sb.tile([128, nm], f32, tag=f"t1{tg}")
            nc.vector.tensor_tensor(out=t1[:], in0=mu[:], in1=mu[:], op=OP.mult)
            nc.vector.tensor_tensor(out=var[:], in0=var[:], in1=t1[:], op=OP.subtract)
            # rstd = 1/sqrt(var+eps)
            for mo in range(nm):
                nc.scalar.activation(out=var[:, mo:mo + 1], in_=var[:, mo:mo + 1],
                                     func=AF.Sqrt, bias=epst[:], scale=1.0)
            nc.vector.reciprocal(out=var[:], in_=var[:])
            # scale = gamma*rstd ; shift = (b - mu)*scale + beta
            nc.vector.tensor_tensor(out=scale[:], in0=pp[:, :, 1], in1=var[:], op=OP.mult)
            nc.vector.tensor_tensor(out=t1[:], in0=pp[:, :, 0], in1=mu[:], op=OP.subtract)
            nc.vector.tensor_tensor(out=t1[:], in0=t1[:], in1=scale[:], op=OP.mult)
            nc.vector.tensor_tensor(out=shift[:], in0=t1[:], in1=pp[:, :, 2], op=OP.add)

        sc1 = sb.tile([128, NM1], f32, tag="sc1")
        sh1 = sb.tile([128, NM1], f32, tag="sh1")
        gn_finalize(sel1[:, 0:NM1, :], adj16, pp1, NM1, P // GROUPS, sc1, sh1, "c1")
        sc2 = sb.tile([128, NM2], f32, tag="sc2")
        sh2 = sb.tile([128, NM2], f32, tag="sh2")
        gn_finalize(sel1[:, NM1:NMS1, :], adj32, pp2, NM2, 2 * P // GROUPS, sc2, sh2, "c2")

        # ---- normalize + relu in place (exact DVE affine + exact ACT relu) ----
        def gn_apply(raw, nm, scale, shift):
            for mo in range(nm):
                nc.vector.tensor_scalar(
                    out=raw[:, mo, :], in0=raw[:, mo, :].bitcast(f32),
                    scalar1=scale[:, mo:mo + 1], scalar2=shift[:, mo:mo + 1],
                    op0=OP.mult, op1=OP.add)
                nc.scalar.activation(out=raw[:, mo, :], in_=raw[:, mo, :].bitcast(f32),
                                     func=AF.Relu)

        gn_apply(raw1, NM1, sc1, sh1)
        gn_apply(raw2, NM2, sc2, sh2)

        # feat output (write normalized conv2 result)
        nc.sync.dma_start(out=feat_d.rearrange("(k p) t -> p k t", p=128),
                          in_=raw2[:].bitcast(f32))

        # ---- conv3 over mixed = [sampled, feat, fm_short] ----
        def rhs3(kc, tt):
            sl = slice(tt * 512, (tt + 1) * 512)
            if kc < NK:
                return smp[:, kc, sl]
            if kc < NK + NM2:
                return raw2[:, kc - NK, sl]
            return raw1[:, kc - NK - NM2, sl]

        raw3 = sb.tile([128, NM3, TL], f32, tag="raw3")
        st3 = sb.tile([128, NM3, NT, 6], f32, tag="st3")
        conv(w3, NK3, NM3, raw3, st3, rhs3)

        if debug:
            nc.sync.dma_start(out=smp_d[:], in_=smp[:].bitcast(f32))
            nc.sync.dma_start(out=raw3_d[:], in_=raw3[:])

        ss3 = sb.tile([128, NM3, 2], f32, tag="ss3")
        chan_stats(st3, pp3, NM3, ss3[:], "c3")
        sel3 = allreduce_stats(ss3, NM3, "3")

        sc3 = sb.tile([128, NM3], f32, tag="sc3")
        sh3 = sb.tile([128, NM3], f32, tag="sh3")
        gn_finalize(sel3[:], adj16, pp3, NM3, C // GROUPS, sc3, sh3, "c3")
        gn_apply(raw3, NM3, sc3, sh3)

        nc.sync.dma_start(out=mixed_d.rearrange("(k p) t -> p k t", p=128), in_=raw3[:])

        ctx.close()

    nc.compile()
    return nc


def _idx_host(frame_level_feature):
    """Bit-exact replication of the reference's inverse-CDF index computation."""
    import jax
    import jax.numpy as jnp
    # the harness passes np.ndarrays, so .mean/.sum run in NUMPY fp32
    # (pairwise summation); only cumsum onwards is jax.  Mirror exactly.
    flf = np.asarray(frame_level_feature, np.float32)
    mean_values = flf.mean(axis=1)[0]
    mean_values = mean_values / mean_values.sum()
    cpu = jax.local_devices(backend="cpu")[0]
    mv = jax.device_put(mean_values, cpu)
    cdf = (jnp.cumsum(mv) * T).astype(jnp.int32)
    cdf = jnp.minimum(cdf, T - 1)
    targets = jax.device_put(np.arange(T, dtype=np.int32), cpu)
    idx = jnp.argmin(jnp.abs(cdf[None, :] - targets[:, None]), axis=1)
    return np.asarray(idx, dtype=np.int64)


def _make_in_maps(inputs, idx):
    feature = np.asarray(inputs["feature"], dtype=np.float32)
    flf_t = np.ascontiguousarray(
        np.asarray(inputs["frame_level_feature"], dtype=np.float32).transpose(0, 2, 1))
    w1t = np.ascontiguousarray(np.asarray(inputs["W1"], np.float32).T)
    w2t = np.ascontiguousarray(np.asarray(inputs["W2"], np.float32).T)
    w3t = np.ascontiguousarray(np.asarray(inputs["W3"], np.float32).T)
    pp1 = np.ascontiguousarray(np.stack([np.asarray(inputs[k], np.float32) for k in ("b1", "g1", "be1")], axis=1))
    pp2 = np.ascontiguousarray(np.stack([np.asarray(inputs[k], np.float32) for k in ("b2", "g2", "be2")], axis=1))
    pp3 = np.ascontiguousarray(np.stack([np.asarray(inputs[k], np.float32) for k in ("b3", "g3", "be3")], axis=1))
    qp = np.arange(128)
    adj16 = (qp[:, None] // 16 == qp[None, :] // 16).astype(np.float32)
    adj32 = (qp[:, None] // 32 == qp[None, :] // 32).astype(np.float32)
    idm = np.eye(128, dtype=np.float32)

    in_maps = []
    for c in range(NCORES):
        b, h = c // 2, c % 2
        sl = idx[h * TL:(h + 1) * TL].astype(np.int16)
        idxw = np.ascontiguousarray(np.tile(sl.reshape(TL // 16, 16).T, (8, 1)))
        oh = np.zeros((128, B), np.float32)
        oh[:, b] = 1.0
        in_maps.append({
            "x": np.ascontiguousarray(feature[b, :, h * TL:(h + 1) * TL]),
            "flf": flf_t[b],
            "idx": idxw,
            "w1t": w1t, "w2t": w2t, "w3t": w3t,
            "pp1": pp1, "pp2": pp2, "pp3": pp3,
            "adj16": adj16, "adj32": adj32, "idm": idm, "oh": oh,
        })
    return in_maps


def kernel(feature, frame_level_feature, W1, b1, g1, be1, W2, b2, g2, be2,
           W3, b3, g3, be3):
    from concourse.bass_utils import run_bass_kernel_spmd

    feature = np.asarray(feature, dtype=np.float32)
    frame_level_feature = np.asarray(frame_level_feature, dtype=np.float32)
    W1, W2, W3 = (np.asarray(w, dtype=np.float32) for w in (W1, W2, W3))
    b1, g1, be1 = (np.asarray(v, dtype=np.float32) for v in (b1, g1, be1))
    b2, g2, be2 = (np.asarray(v, dtype=np.float32) for v in (b2, g2, be2))
    b3, g3, be3 = (np.asarray(v, dtype=np.float32) for v in (b3, g3, be3))

    if "nc" not in _cache:
        _cache["nc"] = _build_nc()
    nc = _cache["nc"]

    idx = _idx_host(frame_level_feature)
    inputs = dict(feature=feature, frame_level_feature=frame_level_feature,
                  W1=W1, b1=b1, g1=g1, be1=be1, W2=W2, b2=b2, g2=g2, be2=be2,
                  W3=W3, b3=b3, g3=g3, be3=be3)
    in_maps = _make_in_maps(inputs, idx)

    res = run_bass_kernel_spmd(nc, in_maps, core_ids=list(range(NCORES)))

    mixed = np.empty((B, C, T), dtype=np.float32)
    feat = np.empty((B, 2 * P, T), dtype=np.float32)
    for c in range(NCORES):
        b, h = c // 2, c % 2
        mixed[b, :, h * TL:(h + 1) * TL] = res.results[c]["mixed"]
        feat[b, :, h * TL:(h + 1) * TL] = res.results[c]["feat"]
    return (mixed, feat)


if __name__ == "__main__":
    import reference
    inputs = {k: np.asarray(v) for k, v in reference.setup_inputs().items()}
    out = kernel(**inputs)
    print([o.shape for o in out])


# revision 8
# speedup vs baseline: 1.1007x; 1.1007x over previous
"""nn_Mixup_Branch kernel for 8 trn2 NeuronCores.

Sharding: core c -> (batch b = c//2, time half h = c%2).  Each core computes
conv1/conv2/GN/ReLU on its (batch, T-half), the inverse-CDF gather from
frame_level_feature, and conv3/GN/ReLU on the concatenated mix.  GroupNorm
statistics need full-T reductions -> two tiny pairwise AllReduces of
per-channel [sum, sumsq].

All matmuls run in float32r (fp32 storage, ~2^-12 effective PE precision).
The gather index vector (2048 int32) is computed on host with the exact jax
ops of the reference (fp32 cumsum rounding determines integer truncation
boundaries, so it must be bit-exact; a 4096-deep sequential fp32 chain is not
reproducible on-device).
"""

import numpy as np

B, C, T, TF, P = 4, 512, 2048, 4096, 512
GROUPS = 32
EPS = 1e-5
TL = T // 2  # per-core time slice
NCORES = 8

_cache = {}


def _build_nc(debug=False):
    import concourse.bacc as bacc
    import concourse.bass as bass
    import concourse.tile as tile
    import concourse.mybir as mybir

    f32 = mybir.dt.float32
    f32r = mybir.dt.float32r
    i16 = mybir.dt.int16
    AF = mybir.ActivationFunctionType
    OP = mybir.AluOpType

    nc = bacc.Bacc("TRN2", num_devices=NCORES)

    # ---- I/O ----
    x_d = nc.dram_tensor("x", [C, TL], f32r, kind="ExternalInput")
    flf_d = nc.dram_tensor("flf", [TF, C], f32r, kind="ExternalInput")
    idx_d = nc.dram_tensor("idx", [128, TL // 16], i16, kind="ExternalInput")
    w1_d = nc.dram_tensor("w1t", [C, P], f32r, kind="ExternalInput")
    w2_d = nc.dram_tensor("w2t", [C, 2 * P], f32r, kind="ExternalInput")
    w3_d = nc.dram_tensor("w3t", [4 * P, C], f32r, kind="ExternalInput")
    # packed per-channel params [bias, gamma, beta] for each conv
    pp1_d = nc.dram_tensor("pp1", [P, 3], f32, kind="ExternalInput")
    pp2_d = nc.dram_tensor("pp2", [2 * P, 3], f32, kind="ExternalInput")
    pp3_d = nc.dram_tensor("pp3", [C, 3], f32, kind="ExternalInput")
    oh_d = nc.dram_tensor("oh", [128, B], f32, kind="ExternalInput")
    adj16_d = nc.dram_tensor("adj16", [128, 128], f32, kind="ExternalInput")
    adj32_d = nc.dram_tensor("adj32", [128, 128], f32, kind="ExternalInput")
    id_d = nc.dram_tensor("idm", [128, 128], f32r, kind="ExternalInput")

    feat_d = nc.dram_tensor("feat", [2 * P, TL], f32, kind="ExternalOutput")
    mixed_d = nc.dram_tensor("mixed", [C, TL], f32, kind="ExternalOutput")
    if debug:
        smp_d = nc.dram_tensor("smp_dbg", [128, 4, TL], f32, kind="ExternalOutput")
        raw3_d = nc.dram_tensor("raw3_dbg", [128, 4, TL], f32, kind="ExternalOutput")

    NK = C // 128          # 4 contraction chunks for conv1/2
    NM1, NM2, NM3 = P // 128, 2 * P // 128, C // 128   # 4, 8, 4
    NK3 = 4 * P // 128     # 16
    NT = TL // 512         # 2 time chunks of 512

    with tile.TileContext(nc) as tc:
        import contextlib
        ctx = contextlib.ExitStack()
        sb = ctx.enter_context(tc.tile_pool(name="sb", bufs=1))
        pcv = ctx.enter_context(tc.tile_pool(name="pcv", bufs=4, space="PSUM"))
        ptp = ctx.enter_context(tc.tile_pool(name="ptp", bufs=4, space="PSUM"))
        dram = ctx.enter_context(tc.tile_pool(name="dram", bufs=1, space="DRAM"))

        # ---- loads: small/critical first, big ones chunked ----
        idxt = sb.tile([128, TL // 16], i16, tag="idx")
        nc.sync.dma_start(out=idxt[:], in_=idx_d[:])
        ident = sb.tile([128, 128], f32r, tag="idm")
        nc.sync.dma_start(out=ident[:], in_=id_d[:])
        adj16 = sb.tile([128, 128], f32, tag="adj16")
        nc.sync.dma_start(out=adj16[:], in_=adj16_d[:])
        adj32 = sb.tile([128, 128], f32, tag="adj32")
        nc.sync.dma_start(out=adj32[:], in_=adj32_d[:])
        oht = sb.tile([128, B], f32, tag="oh")
        nc.sync.dma_start(out=oht[:], in_=oh_d[:])
        pp1 = sb.tile([128, NM1, 3], f32, tag="pp1")
        nc.sync.dma_start(out=pp1[:], in_=pp1_d.rearrange("(m p) v -> p m v", p=128))
        pp2 = sb.tile([128, NM2, 3], f32, tag="pp2")
        nc.sync.dma_start(out=pp2[:], in_=pp2_d.rearrange("(m p) v -> p m v", p=128))
        pp3 = sb.tile([128, NM3, 3], f32, tag="pp3")
        nc.sync.dma_start(out=pp3[:], in_=pp3_d.rearrange("(m p) v -> p m v", p=128))
        epst = sb.tile([128, 1], f32, tag="eps")
        nc.vector.memset(epst[:], EPS)

        xt = sb.tile([128, NK, TL], f32r, tag="x")
        xr = x_d.rearrange("(k p) t -> p k t", p=128)
        w1 = sb.tile([128, NK, NM1, 128], f32r, tag="w1")
        w1r = w1_d.rearrange("(k p) (m q) -> p k m q", p=128, q=128)
        for k in range(NK):
            nc.sync.dma_start(out=xt[:, k, :], in_=xr[:, k, :])
            nc.sync.dma_start(out=w1[:, k, :, :], in_=w1r[:, k, :, :])
        w2 = sb.tile([128, NK, NM2, 128], f32r, tag="w2")
        w2r = w2_d.rearrange("(k p) (m q) -> p k m q", p=128, q=128)
        for k in range(NK):
            nc.sync.dma_start(out=w2[:, k, :, :], in_=w2r[:, k, :, :])
        w3 = sb.tile([128, NK3, NM3, 128], f32r, tag="w3")
        w3r = w3_d.rearrange("(k p) (m q) -> p k m q", p=128, q=128)
        for k in range(0, NK3, 4):
            nc.sync.dma_start(out=w3[:, k:k + 4, :, :], in_=w3r[:, k:k + 4, :, :])

        # ---- gather (after load issue so it doesn't contend with x/w DMAs) ----
        gt = sb.tile([128, TL // 128, C], f32r, tag="gt")
        nc.gpsimd.dma_gather(out_ap=gt[:], in_ap=flf_d[:], idxs_ap=idxt[:],
                             num_idxs=TL, num_idxs_reg=TL, elem_size=C)

        # ---- conv helper: matmuls + psum->sbuf copy + per-tile bn_stats ----
        def conv(w, nk, nm, raw, st6, rhs_of_kc):
            for mo in range(nm):
                for tt in range(NT):
                    ps = pcv.tile([128, 512], f32, tag="cv")
                    for kc in range(nk):
                        nc.tensor.matmul(
                            ps[:], w[:, kc, mo, :], rhs_of_kc(kc, tt),
                            start=(kc == 0), stop=(kc == nk - 1))
                    nc.scalar.copy(out=raw[:, mo, tt * 512:(tt + 1) * 512], in_=ps[:])
                    nc.vector.bn_stats(out=st6[:, mo, tt, :],
                                       in_=raw[:, mo, tt * 512:(tt + 1) * 512].bitcast(f32))

        raw1 = sb.tile([128, NM1, TL], f32r, tag="raw1")
        st1 = sb.tile([128, NM1, NT, 6], f32, tag="st1")
        conv(w1, NK, NM1, raw1, st1, lambda kc, tt: xt[:, kc, tt * 512:(tt + 1) * 512])

        raw2 = sb.tile([128, NM2, TL], f32r, tag="raw2")
        st2 = sb.tile([128, NM2, NT, 6], f32, tag="st2")
        conv(w2, NK, NM2, raw2, st2, lambda kc, tt: xt[:, kc, tt * 512:(tt + 1) * 512])

        # ---- per-channel [sum', sumsq'] with bias folded in ----
        def chan_stats(st6, pp, nm, out_ss, tg):
            # bn_aggr per output-chunk: (128, NT, 6) -> (128, 2) [mean, var]
            mv = sb.tile([128, nm, 2], f32, tag=f"mv{tg}")
            for mo in range(nm):
                nc.vector.bn_aggr(out=mv[:, mo, :], in_=st6[:, mo, :, :])
            # sum = mean*TL + b*TL ; sumsq = (var+mean^2)*TL + 2b*sum_mm + b^2*TL
            sm_mm = sb.tile([128, nm], f32, tag=f"smm{tg}")
            nc.vector.tensor_scalar(out=sm_mm[:], in0=mv[:, :, 0], scalar1=float(TL),
                                    scalar2=None, op0=OP.mult)
            m2 = sb.tile([128, nm], f32, tag=f"m2{tg}")
            nc.vector.tensor_tensor(out=m2[:], in0=mv[:, :, 0], in1=mv[:, :, 0], op=OP.mult)
            nc.vector.tensor_tensor(out=m2[:], in0=m2[:], in1=mv[:, :, 1], op=OP.add)
            # m2 now = mean^2 + var ; sumsq_mm = m2*TL
            nc.vector.tensor_scalar(out=m2[:], in0=m2[:], scalar1=float(TL),
                                    scalar2=None, op0=OP.mult)
            # bias terms
            bcol = pp[:, :, 0]
            t0 = sb.tile([128, nm], f32, tag=f"t0{tg}")
            nc.vector.tensor_tensor(out=t0[:], in0=bcol, in1=sm_mm[:], op=OP.mult)
            nc.vector.tensor_scalar(out=t0[:], in0=t0[:], scalar1=2.0, scalar2=None, op0=OP.mult)
            nc.vector.tensor_tensor(out=m2[:], in0=m2[:], in1=t0[:], op=OP.add)
            nc.vector.tensor_tensor(out=t0[:], in0=bcol, in1=bcol, op=OP.mult)
            nc.vector.tensor_scalar(out=t0[:], in0=t0[:], scalar1=float(TL), scalar2=None, op0=OP.mult)
            nc.vector.tensor_tensor(out=m2[:], in0=m2[:], in1=t0[:], op=OP.add)
            # sum' = sum_mm + b*TL
            nc.vector.tensor_scalar(out=t0[:], in0=bcol, scalar1=float(TL), scalar2=None, op0=OP.mult)
            nc.vector.tensor_tensor(out=out_ss[:, :, 0], in0=t0[:], in1=sm_mm[:], op=OP.add)
            nc.vector.tensor_copy(out=out_ss[:, :, 1], in_=m2[:])

        NMS1 = NM1 + NM2
        ss1 = sb.tile([128, NMS1, 2], f32, tag="ss1")
        chan_stats(st1, pp1, NM1, ss1[:, 0:NM1, :], "c1")
        chan_stats(st2, pp2, NM2, ss1[:, NM1:NMS1, :], "c2")

        def allreduce_stats(ss, nms, tg):
            # pad into per-batch slots via one-hot, one 8-wide AllReduce,
            # then select own slot back out (pairwise groups serialize on
            # the collective cores; a single 8-wide group does not).
            pay = sb.tile([128, B, nms, 2], f32, tag=f"pay{tg}")
            for s in range(B):
                nc.vector.tensor_scalar(out=pay[:, s, :, :], in0=ss[:],
                                        scalar1=oht[:, s:s + 1], scalar2=None, op0=OP.mult)
            ar_in = dram.tile([128, B, nms, 2], f32)
            ar_out = dram.tile([128, B, nms, 2], f32)
            nc.sync.dma_start(out=ar_in[:], in_=pay[:])
            nc.gpsimd.collective_compute(
                "AllReduce", OP.add,
                replica_groups=[[0, 1, 2, 3, 4, 5, 6, 7]],
                ins=[ar_in.opt()], outs=[ar_out.opt()])
            back = sb.tile([128, B, nms, 2], f32, tag=f"back{tg}")
            nc.sync.dma_start(out=back[:], in_=ar_out[:])
            sel = sb.tile([128, nms, 2], f32, tag=f"sel{tg}")
            tmp = sb.tile([128, nms, 2], f32, tag=f"seltmp{tg}")
            nc.vector.tensor_scalar(out=sel[:], in0=back[:, 0, :, :],
                                    scalar1=oht[:, 0:1], scalar2=None, op0=OP.mult)
            for s in range(1, B):
                nc.vector.tensor_scalar(out=tmp[:], in0=back[:, s, :, :],
                                        scalar1=oht[:, s:s + 1], scalar2=None, op0=OP.mult)
                nc.vector.tensor_tensor(out=sel[:], in0=sel[:], in1=tmp[:], op=OP.add)
            return sel

        sel1 = allreduce_stats(ss1, NMS1, "1")

        # ---- transpose gathered rows to channel-major while AR runs ----
        smp = sb.tile([128, NK, TL], f32r, tag="smp")
        for blk in range(TL // 128):
            for kb in range(NK):
                pt = ptp.tile([128, 128], f32r, tag="tp")
                nc.tensor.transpose(pt[:], gt[:, blk, kb * 128:(kb + 1) * 128], ident[:])
                nc.vector.tensor_copy(out=smp[:, kb, blk * 128:(blk + 1) * 128], in_=pt[:])

        # ---- GN finalize: group reduce via adjacency matmul, scale/shift ----
        def gn_finalize(sel_ap, adj, pp, nm, ngel, scale, shift, tg):
            gs = sb.tile([128, nm, 2], f32, tag=f"gs{tg}")
            for mo in range(nm):
                pg = ptp.tile([128, 2], f32, tag="tp")
                nc.tensor.matmul(pg[:], adj[:], sel_ap[:, mo, :],
                                 start=True, stop=True)
                nc.scalar.copy(out=gs[:, mo, :], in_=pg[:])
            ninv = 1.0 / float(ngel * T)
            mu = sb.tile([128, nm], f32, tag=f"mu{tg}")
            nc.vector.tensor_scalar(out=mu[:], in0=gs[:, :, 0], scalar1=ninv, scalar2=None, op0=OP.mult)
            var = sb.tile([128, nm], f32, tag=f"va{tg}")
            nc.vector.tensor_scalar(out=var[:], in0=gs[:, :, 1], scalar1=ninv, scalar2=None, op0=OP.mult)
            t1 = sb.tile([128, nm], f32, tag=f"t1{tg}")
            nc.vector.tensor_tensor(out=t1[:], in0=mu[:], in1=mu[:], op=OP.mult)
            nc.vector.tensor_tensor(out=var[:], in0=var[:], in1=t1[:], op=OP.subtract)
            # rstd = 1/sqrt(var+eps)
            for mo in range(nm):
                nc.scalar.activation(out=var[:, mo:mo + 1], in_=var[:, mo:mo + 1],
                                     func=AF.Sqrt, bias=epst[:], scale=1.0)
            nc.vector.reciprocal(out=var[:], in_=var[:])
            # scale = gamma*rstd ; shift = (b - mu)*scale + beta
            nc.vector.tensor_tensor(out=scale[:], in0=pp[:, :, 1], in1=var[:], op=OP.mult)
            nc.vector.tensor_tensor(out=t1[:], in0=pp[:, :, 0], in1=mu[:], op=OP.subtract)
            nc.vector.tensor_tensor(out=t1[:], in0=t1[:], in1=scale[:], op=OP.mult)
            nc.vector.tensor_tensor(out=shift[:], in0=t1[:], in1=pp[:, :, 2], op=OP.add)

        sc1 = sb.tile([128, NM1], f32, tag="sc1")
        sh1 = sb.tile([128, NM1], f32, tag="sh1")
        gn_finalize(sel1[:, 0:NM1, :], adj16, pp1, NM1, P // GROUPS, sc1, sh1, "c1")
        sc2 = sb.tile([128, NM2], f32, tag="sc2")
        sh2 = sb.tile([128, NM2], f32, tag="sh2")
        gn_finalize(sel1[:, NM1:NMS1, :], adj32, pp2, NM2, 2 * P // GROUPS, sc2, sh2, "c2")

        # ---- normalize + relu in place (exact DVE affine + exact ACT relu) ----
        def gn_apply(raw, nm, scale, shift):
            for mo in range(nm):
                nc.vector.tensor_scalar(
                    out=raw[:, mo, :], in0=raw[:, mo, :].bitcast(f32),
                    scalar1=scale[:, mo:mo + 1], scalar2=shift[:, mo:mo + 1],
                    op0=OP.mult, op1=OP.add)
                nc.scalar.activation(out=raw[:, mo, :], in_=raw[:, mo, :].bitcast(f32),
                                     func=AF.Relu)

        gn_apply(raw1, NM1, sc1, sh1)
        gn_apply(raw2, NM2, sc2, sh2)

        # feat output (write normalized conv2 result)
        nc.sync.dma_start(out=feat_d.rearrange("(k p) t -> p k t", p=128),
                          in_=raw2[:].bitcast(f32))

        # ---- conv3 over mixed = [sampled, feat, fm_short] ----
        def rhs3(kc, tt):
            sl = slice(tt * 512, (tt + 1) * 512)
            if kc < NK:
                return smp[:, kc, sl]
            if kc < NK + NM2:
                return raw2[:, kc - NK, sl]
            return raw1[:, kc - NK - NM2, sl]

        raw3 = sb.tile([128, NM3, TL], f32, tag="raw3")
        st3 = sb.tile([128, NM3, NT, 6], f32, tag="st3")
        conv(w3, NK3, NM3, raw3, st3, rhs3)

        if debug:
            nc.sync.dma_start(out=smp_d[:], in_=smp[:].bitcast(f32))
            nc.sync.dma_start(out=raw3_d[:], in_=raw3[:])

        ss3 = sb.tile([128, NM3, 2], f32, tag="ss3")
        chan_stats(st3, pp3, NM3, ss3[:], "c3")
        sel3 = allreduce_stats(ss3, NM3, "3")

        sc3 = sb.tile([128, NM3], f32, tag="sc3")
        sh3 = sb.tile([128, NM3], f32, tag="sh3")
        gn_finalize(sel3[:], adj16, pp3, NM3, C // GROUPS, sc3, sh3, "c3")
        gn_apply(raw3, NM3, sc3, sh3)

        nc.sync.dma_start(out=mixed_d.rearrange("(k p) t -> p k t", p=128), in_=raw3[:])

        ctx.close()

    nc.compile()
    return nc


def _idx_host(frame_level_feature):
    """Bit-exact replication of the reference's inverse-CDF index computation."""
    import jax
    import jax.numpy as jnp
    # the harness passes np.ndarrays, so .mean/.sum run in NUMPY fp32
    # (pairwise summation); only cumsum onwards is jax.  Mirror exactly.
    flf = np.asarray(frame_level_feature, np.float32)
    mean_values = flf.mean(axis=1)[0]
    mean_values = mean_values / mean_values.sum()
    cpu = jax.local_devices(backend="cpu")[0]
    mv = jax.device_put(mean_values, cpu)
    cdf = (jnp.cumsum(mv) * T).astype(jnp.int32)
    cdf = jnp.minimum(cdf, T - 1)
    targets = jax.device_put(np.arange(T, dtype=np.int32), cpu)
    idx = jnp.argmin(jnp.abs(cdf[None, :] - targets[:, None]), axis=1)
    return np.asarray(idx, dtype=np.int64)


def _make_in_maps(inputs, idx):
    feature = np.asarray(inputs["feature"], dtype=np.float32)
    flf_t = np.ascontiguousarray(
        np.asarray(inputs["frame_level_feature"], dtype=np.float32).transpose(0, 2, 1))
    w1t = np.ascontiguousarray(np.asarray(inputs["W1"], np.float32).T)
    w2t = np.ascontiguousarray(np.asarray(inputs["W2"], np.float32).T)
    w3t = np.ascontiguousarray(np.asarray(inputs["W3"], np.float32).T)
    pp1 = np.ascontiguousarray(np.stack([np.asarray(inputs[k], np.float32) for k in ("b1", "g1", "be1")], axis=1))
    pp2 = np.ascontiguousarray(np.stack([np.asarray(inputs[k], np.float32) for k in ("b2", "g2", "be2")], axis=1))
    pp3 = np.ascontiguousarray(np.stack([np.asarray(inputs[k], np.float32) for k in ("b3", "g3", "be3")], axis=1))
    qp = np.arange(128)
    adj16 = (qp[:, None] // 16 == qp[None, :] // 16).astype(np.float32)
    adj32 = (qp[:, None] // 32 == qp[None, :] // 32).astype(np.float32)
    idm = np.eye(128, dtype=np.float32)

    in_maps = []
    for c in range(NCORES):
        b, h = c // 2, c % 2
        sl = idx[h * TL:(h + 1) * TL].astype(np.int16)
        idxw = np.ascontiguousarray(np.tile(sl.reshape(TL // 16, 16).T, (8, 1)))
        oh = np.zeros((128, B), np.float32)
        oh[:, b] = 1.0
        in_maps.append({
            "x": np.ascontiguousarray(feature[b, :, h * TL:(h + 1) * TL]),
            "flf": flf_t[b],
            "idx": idxw,
            "w1t": w1t, "w2t": w2t, "w3t": w3t,
            "pp1": pp1, "pp2": pp2, "pp3": pp3,
            "adj16": adj16, "adj32": adj32, "idm": idm, "oh": oh,
        })
    return in_maps


def kernel(feature, frame_level_feature, W1, b1, g1, be1, W2, b2, g2, be2,
           W3, b3, g3, be3):
    from concourse.bass_utils import run_bass_kernel_spmd

    feature = np.asarray(feature, dtype=np.float32)
    frame_level_feature = np.asarray(frame_level_feature, dtype=np.float32)
    W1, W2, W3 = (np.asarray(w, dtype=np.float32) for w in (W1, W2, W3))
    b1, g1, be1 = (np.asarray(v, dtype=np.float32) for v in (b1, g1, be1))
    b2, g2, be2 = (np.asarray(v, dtype=np.float32) for v in (b2, g2, be2))
    b3, g3, be3 = (np.asarray(v, dtype=np.float32) for v in (b3, g3, be3))

    if "nc" not in _cache:
        _cache["nc"] = _build_nc()
    nc = _cache["nc"]

    idx = _idx_host(frame_level_feature)
    inputs = dict(feature=feature, frame_level_feature=frame_level_feature,
                  W1=W1, b1=b1, g1=g1, be1=be1, W2=W2, b2=b2, g2=g2, be2=be2,
                  W3=W3, b3=b3, g3=g3, be3=be3)
    in_maps = _make_in_maps(inputs, idx)

    res = run_bass_kernel_spmd(nc, in_maps, core_ids=list(range(NCORES)))

    mixed = np.empty((B, C, T), dtype=np.float32)
    feat = np.empty((B, 2 * P, T), dtype=np.float32)
    for c in range(NCORES):
        b, h = c // 2, c % 2
        mixed[b, :, h * TL:(h + 1) * TL] = res.results[c]["mixed"]
        feat[b, :, h * TL:(h + 1) * TL] = res.results[c]["feat"]
    return (mixed, feat)


if __name__ == "__main__":
    import reference
    inputs = {k: np.asarray(v) for k, v in reference.setup_inputs().items()}
    out = kernel(**inputs)
    print([o.shape for o in out])


# revision 10
# speedup vs baseline: 1.1716x; 1.0644x over previous
"""nn_Mixup_Branch kernel for 8 trn2 NeuronCores.

Sharding: core c -> (batch b = c//2, time half h = c%2).  Each core computes
conv1/conv2/GN/ReLU on its (batch, T-half), the inverse-CDF gather from
frame_level_feature, and conv3/GN/ReLU on the concatenated mix.  GroupNorm
statistics need full-T reductions -> two tiny pairwise AllReduces of
per-channel [sum, sumsq].

All matmuls run in float32r (fp32 storage, ~2^-12 effective PE precision).
The gather index vector (2048 int32) is computed on host with the exact jax
ops of the reference (fp32 cumsum rounding determines integer truncation
boundaries, so it must be bit-exact; a 4096-deep sequential fp32 chain is not
reproducible on-device).
"""

import numpy as np

B, C, T, TF, P = 4, 512, 2048, 4096, 512
GROUPS = 32
EPS = 1e-5
TL = T // 2  # per-core time slice
NCORES = 8

_cache = {}


def _build_nc(debug=False):
    import concourse.bacc as bacc
    import concourse.bass as bass
    import concourse.tile as tile
    import concourse.mybir as mybir

    f32 = mybir.dt.float32
    f32r = mybir.dt.float32r
    i16 = mybir.dt.int16
    AF = mybir.ActivationFunctionType
    OP = mybir.AluOpType

    nc = bacc.Bacc("TRN2", num_devices=NCORES)

    # ---- I/O ----
    x_d = nc.dram_tensor("x", [C, TL], f32r, kind="ExternalInput")
    flf_d = nc.dram_tensor("flf", [TF, C], f32r, kind="ExternalInput")
    idx_d = nc.dram_tensor("idx", [128, TL // 16], i16, kind="ExternalInput")
    w1_d = nc.dram_tensor("w1t", [C, P], f32r, kind="ExternalInput")
    w2_d = nc.dram_tensor("w2t", [C, 2 * P], f32r, kind="ExternalInput")
    w3_d = nc.dram_tensor("w3t", [4 * P, C], f32r, kind="ExternalInput")
    # packed per-channel params [bias, gamma, beta] for each conv
    pp1_d = nc.dram_tensor("pp1", [P, 3], f32, kind="ExternalInput")
    pp2_d = nc.dram_tensor("pp2", [2 * P, 3], f32, kind="ExternalInput")
    pp3_d = nc.dram_tensor("pp3", [C, 3], f32, kind="ExternalInput")
    oh_d = nc.dram_tensor("oh", [128, B], f32, kind="ExternalInput")
    adj16_d = nc.dram_tensor("adj16", [128, 128], f32, kind="ExternalInput")
    adj32_d = nc.dram_tensor("adj32", [128, 128], f32, kind="ExternalInput")
    id_d = nc.dram_tensor("idm", [128, 128], f32r, kind="ExternalInput")

    feat_d = nc.dram_tensor("feat", [2 * P, TL], f32, kind="ExternalOutput")
    mixed_d = nc.dram_tensor("mixed", [C, TL], f32, kind="ExternalOutput")
    if debug:
        smp_d = nc.dram_tensor("smp_dbg", [128, 4, TL], f32, kind="ExternalOutput")
        raw3_d = nc.dram_tensor("raw3_dbg", [128, 4, TL], f32, kind="ExternalOutput")

    NK = C // 128          # 4 contraction chunks for conv1/2
    NM1, NM2, NM3 = P // 128, 2 * P // 128, C // 128   # 4, 8, 4
    NK3 = 4 * P // 128     # 16
    NT = TL // 512         # 2 time chunks of 512

    with tile.TileContext(nc) as tc:
        import contextlib
        ctx = contextlib.ExitStack()
        sb = ctx.enter_context(tc.tile_pool(name="sb", bufs=1))
        pcv = ctx.enter_context(tc.tile_pool(name="pcv", bufs=6, space="PSUM"))
        ptp = ctx.enter_context(tc.tile_pool(name="ptp", bufs=2, space="PSUM"))
        dram = ctx.enter_context(tc.tile_pool(name="dram", bufs=1, space="DRAM"))

        # ---- loads: small/critical first, big ones chunked ----
        idxt = sb.tile([128, TL // 16], i16, tag="idx")
        nc.sync.dma_start(out=idxt[:], in_=idx_d[:])
        ident = sb.tile([128, 128], f32r, tag="idm")
        nc.sync.dma_start(out=ident[:], in_=id_d[:])
        adj16 = sb.tile([128, 128], f32, tag="adj16")
        nc.sync.dma_start(out=adj16[:], in_=adj16_d[:])
        adj32 = sb.tile([128, 128], f32, tag="adj32")
        nc.sync.dma_start(out=adj32[:], in_=adj32_d[:])
        oht = sb.tile([128, B], f32, tag="oh")
        nc.sync.dma_start(out=oht[:], in_=oh_d[:])
        pp1 = sb.tile([128, NM1, 3], f32, tag="pp1")
        nc.sync.dma_start(out=pp1[:], in_=pp1_d.rearrange("(m p) v -> p m v", p=128))
        pp2 = sb.tile([128, NM2, 3], f32, tag="pp2")
        nc.sync.dma_start(out=pp2[:], in_=pp2_d.rearrange("(m p) v -> p m v", p=128))
        pp3 = sb.tile([128, NM3, 3], f32, tag="pp3")
        nc.sync.dma_start(out=pp3[:], in_=pp3_d.rearrange("(m p) v -> p m v", p=128))
        epst = sb.tile([128, 1], f32, tag="eps")
        nc.vector.memset(epst[:], EPS)

        xt = sb.tile([128, NK, TL], f32r, tag="x")
        xr = x_d.rearrange("(k p) t -> p k t", p=128)
        w1 = sb.tile([128, NK, NM1, 128], f32r, tag="w1")
        w1r = w1_d.rearrange("(k p) (m q) -> p k m q", p=128, q=128)
        for k in range(NK):
            nc.sync.dma_start(out=xt[:, k, :], in_=xr[:, k, :])
            nc.sync.dma_start(out=w1[:, k, :, :], in_=w1r[:, k, :, :])
        w2 = sb.tile([128, NK, NM2, 128], f32r, tag="w2")
        w2r = w2_d.rearrange("(k p) (m q) -> p k m q", p=128, q=128)
        for k in range(NK):
            nc.sync.dma_start(out=w2[:, k, :, :], in_=w2r[:, k, :, :])
        w3 = sb.tile([128, NK3, NM3, 128], f32r, tag="w3")
        w3r = w3_d.rearrange("(k p) (m q) -> p k m q", p=128, q=128)
        for k in range(0, NK3, 4):
            nc.sync.dma_start(out=w3[:, k:k + 4, :, :], in_=w3r[:, k:k + 4, :, :])

        # ---- gather (after load issue so it doesn't contend with x/w DMAs) ----
        gt = sb.tile([128, TL // 128, C], f32r, tag="gt")
        nc.gpsimd.dma_gather(out_ap=gt[:], in_ap=flf_d[:], idxs_ap=idxt[:],
                             num_idxs=TL, num_idxs_reg=TL, elem_size=C)

        # ---- conv helper: matmuls + psum->sbuf copy + per-tile bn_stats ----
        def conv(w, nk, nm, raw, st6, rhs_of_kc):
            for mo in range(nm):
                for tt in range(NT):
                    ps = pcv.tile([128, 512], f32, tag="cv")
                    for kc in range(nk):
                        nc.tensor.matmul(
                            ps[:], w[:, kc, mo, :], rhs_of_kc(kc, tt),
                            start=(kc == 0), stop=(kc == nk - 1))
                    nc.scalar.copy(out=raw[:, mo, tt * 512:(tt + 1) * 512], in_=ps[:])
                    nc.vector.bn_stats(out=st6[:, mo, tt, :],
                                       in_=raw[:, mo, tt * 512:(tt + 1) * 512].bitcast(f32))

        # ---- per-channel [sum', sumsq'] with bias folded in ----
        def chan_stats(st6, pp, nm, out_ss, tg):
            # bn_aggr per output-chunk: (128, NT, 6) -> (128, 2) [mean, var]
            mv = sb.tile([128, nm, 2], f32, tag=f"mv{tg}")
            for mo in range(nm):
                nc.vector.bn_aggr(out=mv[:, mo, :], in_=st6[:, mo, :, :])
            # sum = mean*TL + b*TL ; sumsq = (var+mean^2)*TL + 2b*sum_mm + b^2*TL
            sm_mm = sb.tile([128, nm], f32, tag=f"smm{tg}")
            nc.vector.tensor_scalar(out=sm_mm[:], in0=mv[:, :, 0], scalar1=float(TL),
                                    scalar2=None, op0=OP.mult)
            m2 = sb.tile([128, nm], f32, tag=f"m2{tg}")
            nc.vector.tensor_tensor(out=m2[:], in0=mv[:, :, 0], in1=mv[:, :, 0], op=OP.mult)
            nc.vector.tensor_tensor(out=m2[:], in0=m2[:], in1=mv[:, :, 1], op=OP.add)
            # m2 now = mean^2 + var ; sumsq_mm = m2*TL
            nc.vector.tensor_scalar(out=m2[:], in0=m2[:], scalar1=float(TL),
                                    scalar2=None, op0=OP.mult)
            # bias terms
            bcol = pp[:, :, 0]
            t0 = sb.tile([128, nm], f32, tag=f"t0{tg}")
            nc.vector.tensor_tensor(out=t0[:], in0=bcol, in1=sm_mm[:], op=OP.mult)
            nc.vector.tensor_scalar(out=t0[:], in0=t0[:], scalar1=2.0, scalar2=None, op0=OP.mult)
            nc.vector.tensor_tensor(out=m2[:], in0=m2[:], in1=t0[:], op=OP.add)
            nc.vector.tensor_tensor(out=t0[:], in0=bcol, in1=bcol, op=OP.mult)
            nc.vector.tensor_scalar(out=t0[:], in0=t0[:], scalar1=float(TL), scalar2=None, op0=OP.mult)
            nc.vector.tensor_tensor(out=m2[:], in0=m2[:], in1=t0[:], op=OP.add)
            # sum' = sum_mm + b*TL
            nc.vector.tensor_scalar(out=t0[:], in0=bcol, scalar1=float(TL), scalar2=None, op0=OP.mult)
            nc.vector.tensor_tensor(out=out_ss[:, :, 0], in0=t0[:], in1=sm_mm[:], op=OP.add)
            nc.vector.tensor_copy(out=out_ss[:, :, 1], in_=m2[:])


        def allreduce_stats(ss, nms, tg):
            # pad into per-batch slots via one-hot, one 8-wide AllReduce,
            # then select own slot back out (pairwise groups serialize on
            # the collective cores; a single 8-wide group does not).
            pay = sb.tile([128, B, nms, 2], f32, tag=f"pay{tg}")
            for s in range(B):
                nc.vector.tensor_scalar(out=pay[:, s, :, :], in0=ss[:],
                                        scalar1=oht[:, s:s + 1], scalar2=None, op0=OP.mult)
            ar_in = dram.tile([128, B, nms, 2], f32)
            ar_out = dram.tile([128, B, nms, 2], f32)
            nc.sync.dma_start(out=ar_in[:], in_=pay[:])
            nc.gpsimd.collective_compute(
                "AllReduce", OP.add,
                replica_groups=[[0, 1, 2, 3, 4, 5, 6, 7]],
                ins=[ar_in.opt()], outs=[ar_out.opt()])
            back = sb.tile([128, B, nms, 2], f32, tag=f"back{tg}")
            nc.sync.dma_start(out=back[:], in_=ar_out[:])
            sel = sb.tile([128, nms, 2], f32, tag=f"sel{tg}")
            tmp = sb.tile([128, nms, 2], f32, tag=f"seltmp{tg}")
            nc.vector.tensor_scalar(out=sel[:], in0=back[:, 0, :, :],
                                    scalar1=oht[:, 0:1], scalar2=None, op0=OP.mult)
            for s in range(1, B):
                nc.vector.tensor_scalar(out=tmp[:], in0=back[:, s, :, :],
                                        scalar1=oht[:, s:s + 1], scalar2=None, op0=OP.mult)
                nc.vector.tensor_tensor(out=sel[:], in0=sel[:], in1=tmp[:], op=OP.add)
            return sel


        raw1 = sb.tile([128, NM1, TL], f32r, tag="raw1")
        st1 = sb.tile([128, NM1, NT, 6], f32, tag="st1")
        conv(w1, NK, NM1, raw1, st1, lambda kc, tt: xt[:, kc, tt * 512:(tt + 1) * 512])
        ss1 = sb.tile([128, NM1, 2], f32, tag="ss1")
        chan_stats(st1, pp1, NM1, ss1[:], "c1")
        sel1 = allreduce_stats(ss1, NM1, "1")


        # conv1 stats + AR-a issued immediately so the collective hides under conv2
        raw2 = sb.tile([128, NM2, TL], f32r, tag="raw2")
        st2 = sb.tile([128, NM2, NT, 6], f32, tag="st2")
        conv(w2, NK, NM2, raw2, st2, lambda kc, tt: xt[:, kc, tt * 512:(tt + 1) * 512])

        ss2 = sb.tile([128, NM2, 2], f32, tag="ss2")
        chan_stats(st2, pp2, NM2, ss2[:], "c2")
        sel2 = allreduce_stats(ss2, NM2, "2")

        # ---- transpose gathered rows to channel-major while AR runs ----
        smp = sb.tile([128, NK, TL], f32r, tag="smp")
        for blk in range(TL // 128):
            for kb in range(NK):
                pt = ptp.tile([128, 128], f32r, tag="tp")
                nc.tensor.transpose(pt[:], gt[:, blk, kb * 128:(kb + 1) * 128], ident[:])
                nc.vector.tensor_copy(out=smp[:, kb, blk * 128:(blk + 1) * 128], in_=pt[:])

        # ---- GN finalize: group reduce via adjacency matmul, scale/shift ----
        def gn_finalize(sel_ap, adj, pp, nm, ngel, scale, shift, tg):
            gs = sb.tile([128, nm, 2], f32, tag=f"gs{tg}")
            for mo in range(nm):
                pg = ptp.tile([128, 2], f32, tag="tp")
                nc.tensor.matmul(pg[:], adj[:], sel_ap[:, mo, :],
                                 start=True, stop=True)
                nc.scalar.copy(out=gs[:, mo, :], in_=pg[:])
            ninv = 1.0 / float(ngel * T)
            mu = sb.tile([128, nm], f32, tag=f"mu{tg}")
            nc.vector.tensor_scalar(out=mu[:], in0=gs[:, :, 0], scalar1=ninv, scalar2=None, op0=OP.mult)
            var = sb.tile([128, nm], f32, tag=f"va{tg}")
            nc.vector.tensor_scalar(out=var[:], in0=gs[:, :, 1], scalar1=ninv, scalar2=None, op0=OP.mult)
            t1 = sb.tile([128, nm], f32, tag=f"t1{tg}")
            nc.vector.tensor_tensor(out=t1[:], in0=mu[:], in1=mu[:], op=OP.mult)
            nc.vector.tensor_tensor(out=var[:], in0=var[:], in1=t1[:], op=OP.subtract)
            # rstd = 1/sqrt(var+eps)
            for mo in range(nm):
                nc.scalar.activation(out=var[:, mo:mo + 1], in_=var[:, mo:mo + 1],
                                     func=AF.Sqrt, bias=epst[:], scale=1.0)
            nc.vector.reciprocal(out=var[:], in_=var[:])
            # scale = gamma*rstd ; shift = (b - mu)*scale + beta
            nc.vector.tensor_tensor(out=scale[:], in0=pp[:, :, 1], in1=var[:], op=OP.mult)
            nc.vector.tensor_tensor(out=t1[:], in0=pp[:, :, 0], in1=mu[:], op=OP.subtract)
            nc.vector.tensor_tensor(out=t1[:], in0=t1[:], in1=scale[:], op=OP.mult)
            nc.vector.tensor_tensor(out=shift[:], in0=t1[:], in1=pp[:, :, 2], op=OP.add)

        sc1 = sb.tile([128, NM1], f32, tag="sc1")
        sh1 = sb.tile([128, NM1], f32, tag="sh1")
        gn_finalize(sel1[:], adj16, pp1, NM1, P // GROUPS, sc1, sh1, "c1")

        # ---- normalize + relu in place (exact DVE affine + exact ACT relu) ----
        def gn_apply(raw, nm, scale, shift):
            for mo in range(nm):
                nc.vector.tensor_scalar(
                    out=raw[:, mo, :], in0=raw[:, mo, :].bitcast(f32),
                    scalar1=scale[:, mo:mo + 1], scalar2=shift[:, mo:mo + 1],
                    op0=OP.mult, op1=OP.add)
                nc.scalar.activation(out=raw[:, mo, :], in_=raw[:, mo, :].bitcast(f32),
                                     func=AF.Relu)

        gn_apply(raw1, NM1, sc1, sh1)

        # ---- conv3 over mixed, contraction order [sampled, fm, feat-last] so
        # phase A (smp+fm) runs while AR-b is in flight; feat chunks in phase B.
        def rhs3(kc, tt):
            sl = slice(tt * 512, (tt + 1) * 512)
            if kc < NK:
                return smp[:, kc, sl]
            if kc < NK + NM2:
                return raw2[:, kc - NK, sl]
            return raw1[:, kc - NK - NM2, sl]

        KC_EARLY = list(range(NK)) + list(range(NK + NM2, NK3))   # smp + fm
        KC_LATE = list(range(NK, NK + NM2))                        # feat
        raw3 = sb.tile([128, NM3, TL], f32, tag="raw3")
        st3 = sb.tile([128, NM3, NT, 6], f32, tag="st3")
        groups = [(mo, tt) for mo in range(NM3) for tt in range(NT)]
        heldn = 6
        ps3 = {}
        for (mo, tt) in groups[:heldn]:
            ps = pcv.tile([128, 512], f32, tag="cv")
            ps3[(mo, tt)] = ps
            for j, kc in enumerate(KC_EARLY):
                nc.tensor.matmul(ps[:], w3[:, kc, mo, :], rhs3(kc, tt),
                                 start=(j == 0), stop=False)

        sc2 = sb.tile([128, NM2], f32, tag="sc2")
        sh2 = sb.tile([128, NM2], f32, tag="sh2")
        gn_finalize(sel2[:], adj32, pp2, NM2, 2 * P // GROUPS, sc2, sh2, "c2")
        gn_apply(raw2, NM2, sc2, sh2)
        nc.sync.dma_start(out=feat_d.rearrange("(k p) t -> p k t", p=128),
                          in_=raw2[:].bitcast(f32))

        for (mo, tt) in groups[:heldn]:
            ps = ps3[(mo, tt)]
            for j, kc in enumerate(KC_LATE):
                nc.tensor.matmul(ps[:], w3[:, kc, mo, :], rhs3(kc, tt),
                                 start=False, stop=(j == len(KC_LATE) - 1))
            nc.scalar.copy(out=raw3[:, mo, tt * 512:(tt + 1) * 512], in_=ps[:])
            nc.vector.bn_stats(out=st3[:, mo, tt, :],
                               in_=raw3[:, mo, tt * 512:(tt + 1) * 512])
        for (mo, tt) in groups[heldn:]:
            ps = pcv.tile([128, 512], f32, tag="cv")
            order = KC_EARLY + KC_LATE
            for j, kc in enumerate(order):
                nc.tensor.matmul(ps[:], w3[:, kc, mo, :], rhs3(kc, tt),
                                 start=(j == 0), stop=(j == len(order) - 1))
            nc.scalar.copy(out=raw3[:, mo, tt * 512:(tt + 1) * 512], in_=ps[:])
            nc.vector.bn_stats(out=st3[:, mo, tt, :],
                               in_=raw3[:, mo, tt * 512:(tt + 1) * 512])

        if debug:
            nc.sync.dma_start(out=smp_d[:], in_=smp[:].bitcast(f32))
            nc.sync.dma_start(out=raw3_d[:], in_=raw3[:])

        ss3 = sb.tile([128, NM3, 2], f32, tag="ss3")
        chan_stats(st3, pp3, NM3, ss3[:], "c3")
        sel3 = allreduce_stats(ss3, NM3, "3")

        sc3 = sb.tile([128, NM3], f32, tag="sc3")
        sh3 = sb.tile([128, NM3], f32, tag="sh3")
        gn_finalize(sel3[:], adj16, pp3, NM3, C // GROUPS, sc3, sh3, "c3")
        gn_apply(raw3, NM3, sc3, sh3)

        nc.sync.dma_start(out=mixed_d.rearrange("(k p) t -> p k t", p=128), in_=raw3[:])

        ctx.close()

    nc.compile()
    return nc


def _idx_host(frame_level_feature):
    """Bit-exact replication of the reference's inverse-CDF index computation."""
    import jax
    import jax.numpy as jnp
    # the harness passes np.ndarrays, so .mean/.sum run in NUMPY fp32
    # (pairwise summation); only cumsum onwards is jax.  Mirror exactly.
    flf = np.asarray(frame_level_feature, np.float32)
    mean_values = flf.mean(axis=1)[0]
    mean_values = mean_values / mean_values.sum()
    cpu = jax.local_devices(backend="cpu")[0]
    mv = jax.device_put(mean_values, cpu)
    cdf = (jnp.cumsum(mv) * T).astype(jnp.int32)
    cdf = jnp.minimum(cdf, T - 1)
    targets = jax.device_put(np.arange(T, dtype=np.int32), cpu)
    idx = jnp.argmin(jnp.abs(cdf[None, :] - targets[:, None]), axis=1)
    return np.asarray(idx, dtype=np.int64)


def _make_in_maps(inputs, idx):
    feature = np.asarray(inputs["feature"], dtype=np.float32)
    flf_t = np.ascontiguousarray(
        np.asarray(inputs["frame_level_feature"], dtype=np.float32).transpose(0, 2, 1))
    w1t = np.ascontiguousarray(np.asarray(inputs["W1"], np.float32).T)
    w2t = np.ascontiguousarray(np.asarray(inputs["W2"], np.float32).T)
    w3t = np.ascontiguousarray(np.asarray(inputs["W3"], np.float32).T)
    pp1 = np.ascontiguousarray(np.stack([np.asarray(inputs[k], np.float32) for k in ("b1", "g1", "be1")], axis=1))
    pp2 = np.ascontiguousarray(np.stack([np.asarray(inputs[k], np.float32) for k in ("b2", "g2", "be2")], axis=1))
    pp3 = np.ascontiguousarray(np.stack([np.asarray(inputs[k], np.float32) for k in ("b3", "g3", "be3")], axis=1))
    qp = np.arange(128)
    adj16 = (qp[:, None] // 16 == qp[None, :] // 16).astype(np.float32)
    adj32 = (qp[:, None] // 32 == qp[None, :] // 32).astype(np.float32)
    idm = np.eye(128, dtype=np.float32)

    in_maps = []
    for c in range(NCORES):
        b, h = c // 2, c % 2
        sl = idx[h * TL:(h + 1) * TL].astype(np.int16)
        idxw = np.ascontiguousarray(np.tile(sl.reshape(TL // 16, 16).T, (8, 1)))
        oh = np.zeros((128, B), np.float32)
        oh[:, b] = 1.0
        in_maps.append({
            "x": np.ascontiguousarray(feature[b, :, h * TL:(h + 1) * TL]),
            "flf": flf_t[b],
            "idx": idxw,
            "w1t": w1t, "w2t": w2t, "w3t": w3t,
            "pp1": pp1, "pp2": pp2, "pp3": pp3,
            "adj16": adj16, "adj32": adj32, "idm": idm, "oh": oh,
        })
    return in_maps


def kernel(feature, frame_level_feature, W1, b1, g1, be1, W2, b2, g2, be2,
           W3, b3, g3, be3):
    from concourse.bass_utils import run_bass_kernel_spmd

    feature = np.asarray(feature, dtype=np.float32)
    frame_level_feature = np.asarray(frame_level_feature, dtype=np.float32)
    W1, W2, W3 = (np.asarray(w, dtype=np.float32) for w in (W1, W2, W3))
    b1, g1, be1 = (np.asarray(v, dtype=np.float32) for v in (b1, g1, be1))
    b2, g2, be2 = (np.asarray(v, dtype=np.float32) for v in (b2, g2, be2))
    b3, g3, be3 = (np.asarray(v, dtype=np.float32) for v in (b3, g3, be3))

    if "nc" not in _cache:
        _cache["nc"] = _build_nc()
    nc = _cache["nc"]

    idx = _idx_host(frame_level_feature)
    inputs = dict(feature=feature, frame_level_feature=frame_level_feature,
                  W1=W1, b1=b1, g1=g1, be1=be1, W2=W2, b2=b2, g2=g2, be2=be2,
                  W3=W3, b3=b3, g3=g3, be3=be3)
    in_maps = _make_in_maps(inputs, idx)

    res = run_bass_kernel_spmd(nc, in_maps, core_ids=list(range(NCORES)))

    mixed = np.empty((B, C, T), dtype=np.float32)
    feat = np.empty((B, 2 * P, T), dtype=np.float32)
    for c in range(NCORES):
        b, h = c // 2, c % 2
        mixed[b, :, h * TL:(h + 1) * TL] = res.results[c]["mixed"]
        feat[b, :, h * TL:(h + 1) * TL] = res.results[c]["feat"]
    return (mixed, feat)


if __name__ == "__main__":
    import reference
    inputs = {k: np.asarray(v) for k, v in reference.setup_inputs().items()}
    out = kernel(**inputs)
    print([o.shape for o in out])
